# revision 27
# baseline (speedup 1.0000x reference)
"""ConditionalRealNVP.log_prob Trainium2 kernel (8-core data parallel), v4.

Contract: kernel(**inputs) takes the FULL inputs from setup_inputs() and
returns the FULL [B] float32 output of reference().

Strategy
--------
Pure data parallel over the batch: B=524288 rows -> 8 cores x 65536 rows,
tiles of 512 rows.  Everything is feature-major; no PE transposes, no
partition-packing: the op COUNT is minimized (every engine op costs
~0.3-0.7us in dispatch+semaphores regardless of size).

  - One resident slab [69, BT] per tile: rows 0-63 h, rows 64-67 the four
    x coords, row 68 ones (bf16); ONE DMA from a host-packed array.
    mm1 contracts K=69 with per-layer W1 (zero rows for unused coords).
  - mm3 full-width: lhsT [128, 4] with live coords in cols TRANS[l],
    zeros elsewhere -> st [4, 512] at partitions 0-3.  Keep-coords get
    s=0/t=0 so the uniform update x' = (s+1)x + t is an exact no-op.
  - x master xpk [4, 512] f32 at partitions 0-3 (one 8KB DMA init);
    updated by TWO fused scalar_tensor_tensor ops (linear exp,
    biases folded):  A = (s_raw + (b3s+1))*x ; x' = (t_raw + b3t) + A.
    Bridge to slab rows 64-67 = ONE GPSIMD cross-quadrant cast-copy.
  - logdet: sum_j s_j = g2s @ (W3s.1), one [1,512] matmul per layer into
    the y PSUM bank; log_pz via one K=4 matmul of -0.5*x^2.  +C and all
    bias constants folded into the final PSUM->SBUF tensor_scalar.
  - Activations: ACT does gelu1s (table Gelu), gelu2s (Square w/
    scale+bias) and the t-net gelu2 affine (Identity w/ scale+bias);
    DVE does the t-net gelu1 (quad, 2 ops); GPSIMD squares a2.
"""

import math

import numpy as np

B = 524288
D = 4
CTX = 64
HID = 128
IN = 69  # 64 h-rows + 4 x rows + ones row
L = 4
KEEP = ((0, 1), (1, 2), (2, 3), (0, 3))
TRANS = ((2, 3), (0, 3), (0, 1), (1, 2))
NCORES = 8
R = B // NCORES  # rows per core
BT = 512  # rows per tile
LOG2PI = 1.8378770664093453
OUT_CONST = -0.5 * D * LOG2PI

# gelu(z) ~= (GA*z + GC)^2 - GC^2  (quadratic gelu)
GA = math.sqrt(1.0 / math.sqrt(2.0 * math.pi))
GC = 0.25 / GA

_CACHE = {}

# CoreSim has no table-Gelu; set True (tests only) to swap the s-net gelu1
# to the quadratic Square form so the kernel can run in simulation.
SIM_SAFE_GELU = False


def _build_nc(rows):
    import concourse.tile as tile
    from concourse import bacc, mybir

    dt = mybir.dt
    F32, BF16 = dt.float32, dt.bfloat16
    AF = mybir.ActivationFunctionType
    OP = mybir.AluOpType

    nt = rows // BT

    nc = bacc.Bacc("TRN2")
    slabInit = nc.dram_tensor("slabInit", [IN, rows], BF16, kind="ExternalInput")
    thetaT4 = nc.dram_tensor("thetaT4", [4, rows], F32, kind="ExternalInput")
    w1 = nc.dram_tensor("w1", [2 * L, IN, HID], BF16, kind="ExternalInput")
    w2 = nc.dram_tensor("w2", [2 * L, HID, HID], BF16, kind="ExternalInput")
    w3 = nc.dram_tensor("w3", [2 * L, HID, 4], BF16, kind="ExternalInput")
    w3sum = nc.dram_tensor("w3sum", [HID, L], BF16, kind="ExternalInput")
    actb_s = nc.dram_tensor("actb_s", [HID, L], F32, kind="ExternalInput")
    actb_t = nc.dram_tensor("actb_t", [HID, L], F32, kind="ExternalInput")
    bsp1 = nc.dram_tensor("bsp1", [4, L], F32, kind="ExternalInput")
    btv = nc.dram_tensor("btv", [4, L], F32, kind="ExternalInput")
    csq = nc.dram_tensor("csq", [4, 1], BF16, kind="ExternalInput")
    yconst = nc.dram_tensor("yconst", [1, 1], F32, kind="ExternalInput")
    y = nc.dram_tensor("y", [rows], F32, kind="ExternalOutput")

    TWO_C1 = 2.0 * GC / GA

    with tile.TileContext(nc) as tc:
        with (
            tc.tile_pool(name="singles", bufs=1) as singles,
            tc.tile_pool(name="slabs", bufs=4) as slabp,
            tc.tile_pool(name="state", bufs=4) as state,
            tc.tile_pool(name="work", bufs=4) as work,
            tc.tile_pool(name="hp", bufs=4, space="PSUM") as hp,
            tc.tile_pool(name="stp", bufs=1, space="PSUM") as stp,
            tc.tile_pool(name="yp", bufs=2, space="PSUM") as yp,
        ):
            # ---- resident constants ----
            w1_sb = singles.tile([IN, 2 * L, HID], BF16)
            nc.sync.dma_start(w1_sb[:], w1[:].rearrange("n k m -> k n m"))
            w2_sb = singles.tile([HID, 2 * L, HID], BF16)
            nc.sync.dma_start(w2_sb[:], w2[:].rearrange("n k m -> k n m"))
            w3_sb = singles.tile([HID, 2 * L, 4], BF16)
            nc.sync.dma_start(w3_sb[:], w3[:].rearrange("n k m -> k n m"))
            w3sum_sb = singles.tile([HID, L], BF16)
            nc.sync.dma_start(w3sum_sb[:], w3sum[:])
            actbs_sb = singles.tile([HID, L], F32)
            nc.sync.dma_start(actbs_sb[:], actb_s[:])
            actbt_sb = singles.tile([HID, L], F32)
            nc.sync.dma_start(actbt_sb[:], actb_t[:])
            bsp1_sb = singles.tile([4, L], F32)
            nc.sync.dma_start(bsp1_sb[:], bsp1[:])
            btv_sb = singles.tile([4, L], F32)
            nc.sync.dma_start(btv_sb[:], btv[:])
            csq_sb = singles.tile([4, 1], BF16)
            nc.sync.dma_start(csq_sb[:], csq[:])
            yconst_sb = singles.tile([1, 1], F32)
            nc.sync.dma_start(yconst_sb[:], yconst[:])
            gc_sb = None
            if SIM_SAFE_GELU:
                gc_sb = singles.tile([128, 1], F32)
                nc.vector.memset(gc_sb[:], GC)

            for it in range(nt):
                r0 = it * BT

                slab = slabp.tile([IN, BT], BF16, tag="slab")
                nc.sync.dma_start(slab[:], slabInit[:, r0 : r0 + BT])

                xpk = state.tile([4, BT], F32, tag="xpk")
                nc.sync.dma_start(xpk[:], thetaT4[:, r0 : r0 + BT])

                yps = yp.tile([1, BT], F32, tag="yps")

                for l in range(L):
                    si, ti = 2 * l, 2 * l + 1

                    # ---- mm1 ----
                    h1s = hp.tile([128, BT], F32, tag="h")
                    nc.tensor.matmul(
                        h1s[:], w1_sb[:, si, :], slab[:], start=True, stop=True
                    )
                    h1t = hp.tile([128, BT], F32, tag="h")
                    nc.tensor.matmul(
                        h1t[:], w1_sb[:, ti, :], slab[:], start=True, stop=True
                    )

                    # ---- gelu1: s exact table on ACT; t quadratic on DVE ----
                    g1s = work.tile([128, BT], BF16, tag="g1s")
                    if SIM_SAFE_GELU:
                        nc.scalar.activation(
                            g1s[:], h1s[:], AF.Square, bias=gc_sb[:], scale=GA
                        )
                    else:
                        nc.scalar.activation(g1s[:], h1s[:], AF.Gelu)
                    a1 = work.tile([128, BT], BF16, tag="a1")
                    nc.vector.tensor_scalar(a1[:], h1t[:], TWO_C1, None, OP.add)
                    g1t = work.tile([128, BT], BF16, tag="g1t")
                    nc.vector.scalar_tensor_tensor(
                        g1t[:], a1[:], -TWO_C1, a1[:], OP.add, OP.mult
                    )

                    # ---- mm2 (biases folded into gelu2 affines) ----
                    h2s = hp.tile([128, BT], F32, tag="h")
                    nc.tensor.matmul(
                        h2s[:], w2_sb[:, si, :], g1s[:], start=True, stop=True
                    )
                    h2t = hp.tile([128, BT], F32, tag="h")
                    nc.tensor.matmul(
                        h2t[:], w2_sb[:, ti, :], g1t[:], start=True, stop=True
                    )

                    # ---- gelu2: s ACT Square; t ACT Identity affine + GPSIMD sq ----
                    g2s = work.tile([128, BT], BF16, tag="g2s")
                    nc.scalar.activation(
                        g2s[:], h2s[:], AF.Square,
                        bias=actbs_sb[:, l : l + 1], scale=GA,
                    )
                    a2 = work.tile([128, BT], BF16, tag="a2")
                    nc.scalar.activation(
                        a2[:], h2t[:], AF.Identity,
                        bias=actbt_sb[:, l : l + 1], scale=GA,
                    )
                    g2t = work.tile([128, BT], BF16, tag="g2t")
                    nc.gpsimd.tensor_mul(g2t[:], a2[:], a2[:])

                    # ---- logdet: yps += g2s @ (W3s.1)  (one matmul) ----
                    nc.tensor.matmul(
                        yps[0:1, :], w3sum_sb[:, l : l + 1], g2s[:],
                        start=(l == 0), stop=False, skip_group_check=True,
                    )

                    # ---- mm3: full-width, out [4, BT] at partitions 0-3 ----
                    sts = stp.tile([4, BT], F32, tag="sts")
                    nc.tensor.matmul(
                        sts[:], w3_sb[:, si, :], g2s[:], start=True, stop=True
                    )
                    stt = stp.tile([4, BT], F32, tag="stt")
                    nc.tensor.matmul(
                        stt[:], w3_sb[:, ti, :], g2t[:], start=True, stop=True
                    )

                    # ---- x' = (s_raw + b3s + 1)*x + (t_raw + b3t) ----
                    av = work.tile([4, BT], F32, tag="av")
                    nc.vector.scalar_tensor_tensor(
                        av[:], sts[:], bsp1_sb[:, l : l + 1], xpk[:],
                        OP.add, OP.mult,
                    )
                    nc.vector.scalar_tensor_tensor(
                        xpk[:], stt[:], btv_sb[:, l : l + 1], av[:],
                        OP.add, OP.add,
                    )

                    # ---- bridge x into slab rows 64-67 (cast + quadrant move) ----
                    if l < 3:
                        nc.gpsimd.tensor_copy(slab[64:68, :], xpk[:])

                # ---- tail: yps += -0.5 * x^2; y = yps + const ----
                sq = work.tile([4, BT], BF16, tag="sq")
                nc.gpsimd.tensor_mul(sq[:], xpk[:], xpk[:])
                nc.tensor.matmul(
                    yps[0:1, :], csq_sb[:], sq[:],
                    start=False, stop=True, skip_group_check=True,
                )
                ysb = work.tile([1, BT], F32, tag="ysb")
                nc.vector.tensor_scalar(
                    ysb[:], yps[:], yconst_sb[0:1, 0:1], None, OP.add
                )
                nc.sync.dma_start(
                    y[r0 : r0 + BT].rearrange("(a b) -> a b", a=1), ysb[:]
                )

    nc.compile()
    return nc


def _prep_inputs(theta, h, sW1, sb1, sW2, sb2, sW3, sb3, tW1, tb1, tW2, tb2, tW3, tb3):
    """Host-side packing/folding. Returns dict of full-size arrays."""
    import ml_dtypes

    bf16 = ml_dtypes.bfloat16
    f32 = np.float32
    theta = np.asarray(theta, f32)
    h = np.asarray(h, f32)

    # slab rows: 0-63 h.T, 64-67 theta.T, 68 ones
    slabInit = np.empty((IN, B), bf16)
    slabInit[0:64, :] = np.ascontiguousarray(h.T).astype(bf16)
    slabInit[64:68, :] = np.ascontiguousarray(theta.T).astype(bf16)
    slabInit[68, :] = np.ones((B,), bf16)
    thetaT4 = np.ascontiguousarray(theta.T).astype(f32)

    w1 = np.zeros((2 * L, IN, HID), f32)
    w2 = np.zeros((2 * L, HID, HID), f32)
    w3 = np.zeros((2 * L, HID, 4), f32)
    w3sum = np.zeros((HID, L), f32)
    actb_s = np.zeros((HID, L), f32)
    actb_t = np.zeros((HID, L), f32)
    bsp1 = np.ones((4, L), f32)
    btv = np.zeros((4, L), f32)
    yconst = OUT_CONST
    for i in range(L):
        t0, t1 = TRANS[i]
        for j, (W1, B1, W2_, B2, W3_, B3) in enumerate(
            ((sW1, sb1, sW2, sb2, sW3, sb3), (tW1, tb1, tW2, tb2, tW3, tb3))
        ):
            n = 2 * i + j
            W1i, B1i = np.asarray(W1[i], f32), np.asarray(B1[i], f32)
            W2i, B2i = np.asarray(W2_[i], f32), np.asarray(B2[i], f32)
            W3i, B3i = np.asarray(W3_[i], f32), np.asarray(B3[i], f32)
            # mm1 rows: [h(64); x0..x3 (keep coords only); b1]
            w1[n, 0:64, :] = W1i[2:66]
            k0, k1 = KEEP[i]
            w1[n, 64 + k0, :] = W1i[0]
            w1[n, 64 + k1, :] = W1i[1]
            w1[n, 68, :] = B1i
            # mm2: s-net plain (exact gelu1); t-net folds GA^2 (quad gelu1)
            w2[n] = W2i if j == 0 else (GA * GA) * W2i
            # gelu2 affine constants: beta = GA*b2 + GC
            beta = GA * B2i + GC
            if j == 0:
                actb_s[:, i] = beta
            else:
                actb_t[:, i] = beta
            # mm3: [128, 4], live coords in cols t0/t1
            w3[n, :, t0] = W3i[:, 0]
            w3[n, :, t1] = W3i[:, 1]
            b3eff = B3i - GC * GC * W3i.sum(axis=0)
            if j == 0:
                # logdet pieces: w3sum + bias constants into yconst
                w3sum[:, i] = W3i[:, 0] + W3i[:, 1]
                yconst += b3eff.sum()
                bsp1[t0, i] = b3eff[0] + 1.0
                bsp1[t1, i] = b3eff[1] + 1.0
            else:
                btv[t0, i] = b3eff[0]
                btv[t1, i] = b3eff[1]

    csq = np.full((4, 1), -0.5, f32)

    return {
        "slabInit": slabInit,
        "thetaT4": thetaT4,
        "w1": w1.astype(bf16),
        "w2": w2.astype(bf16),
        "w3": w3.astype(bf16),
        "w3sum": w3sum.astype(bf16),
        "actb_s": actb_s,
        "actb_t": actb_t,
        "bsp1": bsp1,
        "btv": btv,
        "csq": csq.astype(bf16),
        "yconst": np.full((1, 1), yconst, f32),
    }


def _get_nc(rows):
    key = ("nc", rows)
    if key not in _CACHE:
        _CACHE[key] = _build_nc(rows)
    return _CACHE[key]


def _run(inputs, trace=False, rows=R, ncores=NCORES):
    from concourse.bass_utils import run_bass_kernel_spmd

    full = _prep_inputs(**inputs)
    shared = {k: v for k, v in full.items() if k not in ("slabInit", "thetaT4")}
    in_maps = []
    for c in range(ncores):
        r0 = c * rows
        m = dict(shared)
        m["slabInit"] = np.ascontiguousarray(full["slabInit"][:, r0 : r0 + rows])
        m["thetaT4"] = np.ascontiguousarray(full["thetaT4"][:, r0 : r0 + rows])
        in_maps.append(m)

    nc = _get_nc(rows)
    res = run_bass_kernel_spmd(
        nc, in_maps, core_ids=list(range(ncores)), trace=trace
    )
    out = np.concatenate([res.results[c]["y"] for c in range(ncores)])
    return out, res


def kernel(**inputs):
    out, _ = _run(inputs)
    return out.astype(np.float32)


# revision 29
# speedup vs baseline: 1.6023x; 1.6023x over previous
"""ConditionalRealNVP.log_prob Trainium2 kernel (8-core data parallel), v4.

Contract: kernel(**inputs) takes the FULL inputs from setup_inputs() and
returns the FULL [B] float32 output of reference().

Strategy
--------
Pure data parallel over the batch: B=524288 rows -> 8 cores x 65536 rows,
tiles of 512 rows.  Everything is feature-major; no PE transposes, no
partition-packing: the op COUNT is minimized (every engine op costs
~0.3-0.7us in dispatch+semaphores regardless of size).

  - One resident slab [69, BT] per tile: rows 0-63 h, rows 64-67 the four
    x coords, row 68 ones (bf16); ONE DMA from a host-packed array.
    mm1 contracts K=69 with per-layer W1 (zero rows for unused coords).
  - mm3 full-width: lhsT [128, 4] with live coords in cols TRANS[l],
    zeros elsewhere -> st [4, 512] at partitions 0-3.  Keep-coords get
    s=0/t=0 so the uniform update x' = (s+1)x + t is an exact no-op.
  - x master xpk [4, 512] f32 at partitions 0-3 (one 8KB DMA init);
    updated by TWO fused scalar_tensor_tensor ops (linear exp,
    biases folded):  A = (s_raw + (b3s+1))*x ; x' = (t_raw + b3t) + A.
    Bridge to slab rows 64-67 = ONE GPSIMD cross-quadrant cast-copy.
  - logdet: sum_j s_j = g2s @ (W3s.1), one [1,512] matmul per layer into
    the y PSUM bank; log_pz via one K=4 matmul of -0.5*x^2.  +C and all
    bias constants folded into the final PSUM->SBUF tensor_scalar.
  - Activations: ACT does gelu1s (table Gelu), gelu2s (Square w/
    scale+bias) and the t-net gelu2 affine (Identity w/ scale+bias);
    DVE does the t-net gelu1 (quad, 2 ops); GPSIMD squares a2.
"""

import math

import numpy as np

B = 524288
D = 4
CTX = 64
HID = 128
IN = 69  # 64 h-rows + 4 x rows + ones row
L = 4
KEEP = ((0, 1), (1, 2), (2, 3), (0, 3))
TRANS = ((2, 3), (0, 3), (0, 1), (1, 2))
NCORES = 8
R = B // NCORES  # rows per core
BT = 512  # rows per tile
LOG2PI = 1.8378770664093453
OUT_CONST = -0.5 * D * LOG2PI

# gelu(z) ~= (GA*z + GC)^2 - GC^2  (quadratic gelu)
GA = math.sqrt(1.0 / math.sqrt(2.0 * math.pi))
GC = 0.25 / GA

_CACHE = {}

# CoreSim has no table-Gelu; set True (tests only) to swap the s-net gelu1
# to the quadratic Square form so the kernel can run in simulation.
SIM_SAFE_GELU = False


def _build_nc(rows):
    import concourse.tile as tile
    from concourse import bacc, mybir

    dt = mybir.dt
    F32, BF16 = dt.float32, dt.bfloat16
    AF = mybir.ActivationFunctionType
    OP = mybir.AluOpType

    nt = rows // BT

    nc = bacc.Bacc("TRN2")
    slabInit = nc.dram_tensor("slabInit", [IN, rows], BF16, kind="ExternalInput")
    thetaT4 = nc.dram_tensor("thetaT4", [4, rows], F32, kind="ExternalInput")
    w1 = nc.dram_tensor("w1", [2 * L, IN, HID], BF16, kind="ExternalInput")
    w2 = nc.dram_tensor("w2", [2 * L, HID, HID], BF16, kind="ExternalInput")
    w3 = nc.dram_tensor("w3", [2 * L, HID, 4], BF16, kind="ExternalInput")
    w3sum = nc.dram_tensor("w3sum", [HID, L], BF16, kind="ExternalInput")
    actb_s = nc.dram_tensor("actb_s", [HID, L], F32, kind="ExternalInput")
    actb_t = nc.dram_tensor("actb_t", [HID, L], F32, kind="ExternalInput")
    bsp1 = nc.dram_tensor("bsp1", [4, L], F32, kind="ExternalInput")
    btv = nc.dram_tensor("btv", [4, L], F32, kind="ExternalInput")
    csq = nc.dram_tensor("csq", [4, 1], BF16, kind="ExternalInput")
    yconst = nc.dram_tensor("yconst", [1, 1], F32, kind="ExternalInput")
    y = nc.dram_tensor("y", [rows], F32, kind="ExternalOutput")

    TWO_C1 = 2.0 * GC / GA

    with tile.TileContext(nc) as tc:
        with (
            tc.tile_pool(name="singles", bufs=1) as singles,
            tc.tile_pool(name="slabs", bufs=4) as slabp,
            tc.tile_pool(name="state", bufs=4) as state,
            tc.tile_pool(name="work", bufs=4) as work,
            tc.tile_pool(name="hp", bufs=3, space="PSUM") as hp,
            tc.tile_pool(name="stp", bufs=3, space="PSUM") as stp,
            tc.tile_pool(name="yp", bufs=2, space="PSUM") as yp,
        ):
            # ---- resident constants ----
            w1_sb = singles.tile([IN, 2 * L, HID], BF16)
            nc.sync.dma_start(w1_sb[:], w1[:].rearrange("n k m -> k n m"))
            w2_sb = singles.tile([HID, 2 * L, HID], BF16)
            nc.sync.dma_start(w2_sb[:], w2[:].rearrange("n k m -> k n m"))
            w3_sb = singles.tile([HID, 2 * L, 4], BF16)
            nc.sync.dma_start(w3_sb[:], w3[:].rearrange("n k m -> k n m"))
            w3sum_sb = singles.tile([HID, L], BF16)
            nc.sync.dma_start(w3sum_sb[:], w3sum[:])
            actbs_sb = singles.tile([HID, L], F32)
            nc.sync.dma_start(actbs_sb[:], actb_s[:])
            actbt_sb = singles.tile([HID, L], F32)
            nc.sync.dma_start(actbt_sb[:], actb_t[:])
            bsp1_sb = singles.tile([4, L], F32)
            nc.sync.dma_start(bsp1_sb[:], bsp1[:])
            btv_sb = singles.tile([4, L], F32)
            nc.sync.dma_start(btv_sb[:], btv[:])
            csq_sb = singles.tile([4, 1], BF16)
            nc.sync.dma_start(csq_sb[:], csq[:])
            yconst_sb = singles.tile([1, 1], F32)
            nc.sync.dma_start(yconst_sb[:], yconst[:])
            gc_sb = None
            if SIM_SAFE_GELU:
                gc_sb = singles.tile([128, 1], F32)
                nc.vector.memset(gc_sb[:], GC)

            def tile_gen(it):
                """One tile's instruction stream, yielding between stages so
                the driver can interleave several tiles (software pipelining:
                each engine then has independent work to fill dependency
                stalls)."""
                r0 = it * BT

                slab = slabp.tile([IN, BT], BF16, tag="slab")
                nc.sync.dma_start(slab[:], slabInit[:, r0 : r0 + BT])
                xpk = state.tile([4, BT], F32, tag="xpk")
                nc.sync.dma_start(xpk[:], thetaT4[:, r0 : r0 + BT])
                yps = yp.tile([1, BT], F32, tag="yps")
                yield

                for l in range(L):
                    si, ti = 2 * l, 2 * l + 1

                    h1s = hp.tile([128, BT], F32, tag="h")
                    nc.tensor.matmul(
                        h1s[:], w1_sb[:, si, :], slab[:], start=True, stop=True
                    )
                    h1t = hp.tile([128, BT], F32, tag="h")
                    nc.tensor.matmul(
                        h1t[:], w1_sb[:, ti, :], slab[:], start=True, stop=True
                    )
                    yield

                    # gelu1: s exact table on ACT; t quadratic on DVE
                    g1s = work.tile([128, BT], BF16, tag="g1s")
                    if SIM_SAFE_GELU:
                        nc.scalar.activation(
                            g1s[:], h1s[:], AF.Square, bias=gc_sb[:], scale=GA
                        )
                    else:
                        nc.scalar.activation(g1s[:], h1s[:], AF.Gelu)
                    a1 = work.tile([128, BT], BF16, tag="a1")
                    nc.vector.tensor_scalar(a1[:], h1t[:], TWO_C1, None, OP.add)
                    g1t = work.tile([128, BT], BF16, tag="g1t")
                    nc.vector.scalar_tensor_tensor(
                        g1t[:], a1[:], -TWO_C1, a1[:], OP.add, OP.mult
                    )
                    yield

                    h2s = hp.tile([128, BT], F32, tag="h")
                    nc.tensor.matmul(
                        h2s[:], w2_sb[:, si, :], g1s[:], start=True, stop=True
                    )
                    h2t = hp.tile([128, BT], F32, tag="h")
                    nc.tensor.matmul(
                        h2t[:], w2_sb[:, ti, :], g1t[:], start=True, stop=True
                    )
                    yield

                    # gelu2: s ACT Square; t ACT Identity affine + DVE square
                    g2s = work.tile([128, BT], BF16, tag="g2s")
                    nc.scalar.activation(
                        g2s[:], h2s[:], AF.Square,
                        bias=actbs_sb[:, l : l + 1], scale=GA,
                    )
                    a2 = work.tile([128, BT], BF16, tag="a2")
                    nc.scalar.activation(
                        a2[:], h2t[:], AF.Identity,
                        bias=actbt_sb[:, l : l + 1], scale=GA,
                    )
                    g2t = work.tile([128, BT], BF16, tag="g2t")
                    nc.vector.tensor_mul(g2t[:], a2[:], a2[:])
                    yield

                    # logdet matmul + mm3
                    nc.tensor.matmul(
                        yps[0:1, :], w3sum_sb[:, l : l + 1], g2s[:],
                        start=(l == 0), stop=False, skip_group_check=True,
                    )
                    sts = stp.tile([4, BT], F32, tag="st")
                    nc.tensor.matmul(
                        sts[:], w3_sb[:, si, :], g2s[:], start=True, stop=True
                    )
                    stt = stp.tile([4, BT], F32, tag="st")
                    nc.tensor.matmul(
                        stt[:], w3_sb[:, ti, :], g2t[:], start=True, stop=True
                    )
                    yield

                    # x' = (s_raw + b3s + 1)*x + (t_raw + b3t)
                    av = work.tile([4, BT], F32, tag="av")
                    nc.vector.scalar_tensor_tensor(
                        av[:], sts[:], bsp1_sb[:, l : l + 1], xpk[:],
                        OP.add, OP.mult,
                    )
                    nc.vector.scalar_tensor_tensor(
                        xpk[:], stt[:], btv_sb[:, l : l + 1], av[:],
                        OP.add, OP.add,
                    )
                    if l < 3:
                        nc.gpsimd.tensor_copy(slab[64:68, :], xpk[:])
                    yield

                # tail: yps += -0.5 * x^2; y = yps + const
                sq = work.tile([4, BT], BF16, tag="sq")
                nc.gpsimd.tensor_mul(sq[:], xpk[:], xpk[:])
                yield
                nc.tensor.matmul(
                    yps[0:1, :], csq_sb[:], sq[:],
                    start=False, stop=True, skip_group_check=True,
                )
                ysb = work.tile([1, BT], F32, tag="ysb")
                nc.vector.tensor_scalar(
                    ysb[:], yps[:], yconst_sb[0:1, 0:1], None, OP.add
                )
                nc.sync.dma_start(
                    y[r0 : r0 + BT].rearrange("(a b) -> a b", a=1), ysb[:]
                )

            # rolling-window software pipeline over tiles
            from collections import deque

            W = 2
            active = deque()
            next_it = 0
            while active or next_it < nt:
                while len(active) < W and next_it < nt:
                    active.append(tile_gen(next_it))
                    next_it += 1
                g = active.popleft()
                try:
                    next(g)
                    active.append(g)
                except StopIteration:
                    pass

    nc.compile()
    return nc


def _prep_inputs(theta, h, sW1, sb1, sW2, sb2, sW3, sb3, tW1, tb1, tW2, tb2, tW3, tb3):
    """Host-side packing/folding. Returns dict of full-size arrays."""
    import ml_dtypes

    bf16 = ml_dtypes.bfloat16
    f32 = np.float32
    theta = np.asarray(theta, f32)
    h = np.asarray(h, f32)

    # slab rows: 0-63 h.T, 64-67 theta.T, 68 ones
    slabInit = np.empty((IN, B), bf16)
    slabInit[0:64, :] = np.ascontiguousarray(h.T).astype(bf16)
    slabInit[64:68, :] = np.ascontiguousarray(theta.T).astype(bf16)
    slabInit[68, :] = np.ones((B,), bf16)
    thetaT4 = np.ascontiguousarray(theta.T).astype(f32)

    w1 = np.zeros((2 * L, IN, HID), f32)
    w2 = np.zeros((2 * L, HID, HID), f32)
    w3 = np.zeros((2 * L, HID, 4), f32)
    w3sum = np.zeros((HID, L), f32)
    actb_s = np.zeros((HID, L), f32)
    actb_t = np.zeros((HID, L), f32)
    bsp1 = np.ones((4, L), f32)
    btv = np.zeros((4, L), f32)
    yconst = OUT_CONST
    for i in range(L):
        t0, t1 = TRANS[i]
        for j, (W1, B1, W2_, B2, W3_, B3) in enumerate(
            ((sW1, sb1, sW2, sb2, sW3, sb3), (tW1, tb1, tW2, tb2, tW3, tb3))
        ):
            n = 2 * i + j
            W1i, B1i = np.asarray(W1[i], f32), np.asarray(B1[i], f32)
            W2i, B2i = np.asarray(W2_[i], f32), np.asarray(B2[i], f32)
            W3i, B3i = np.asarray(W3_[i], f32), np.asarray(B3[i], f32)
            # mm1 rows: [h(64); x0..x3 (keep coords only); b1]
            w1[n, 0:64, :] = W1i[2:66]
            k0, k1 = KEEP[i]
            w1[n, 64 + k0, :] = W1i[0]
            w1[n, 64 + k1, :] = W1i[1]
            w1[n, 68, :] = B1i
            # mm2: s-net plain (exact gelu1); t-net folds GA^2 (quad gelu1)
            w2[n] = W2i if j == 0 else (GA * GA) * W2i
            # gelu2 affine constants: beta = GA*b2 + GC
            beta = GA * B2i + GC
            if j == 0:
                actb_s[:, i] = beta
            else:
                actb_t[:, i] = beta
            # mm3: [128, 4], live coords in cols t0/t1
            w3[n, :, t0] = W3i[:, 0]
            w3[n, :, t1] = W3i[:, 1]
            b3eff = B3i - GC * GC * W3i.sum(axis=0)
            if j == 0:
                # logdet pieces: w3sum + bias constants into yconst
                w3sum[:, i] = W3i[:, 0] + W3i[:, 1]
                yconst += b3eff.sum()
                bsp1[t0, i] = b3eff[0] + 1.0
                bsp1[t1, i] = b3eff[1] + 1.0
            else:
                btv[t0, i] = b3eff[0]
                btv[t1, i] = b3eff[1]

    csq = np.full((4, 1), -0.5, f32)

    return {
        "slabInit": slabInit,
        "thetaT4": thetaT4,
        "w1": w1.astype(bf16),
        "w2": w2.astype(bf16),
        "w3": w3.astype(bf16),
        "w3sum": w3sum.astype(bf16),
        "actb_s": actb_s,
        "actb_t": actb_t,
        "bsp1": bsp1,
        "btv": btv,
        "csq": csq.astype(bf16),
        "yconst": np.full((1, 1), yconst, f32),
    }


def _get_nc(rows):
    key = ("nc", rows)
    if key not in _CACHE:
        _CACHE[key] = _build_nc(rows)
    return _CACHE[key]


def _run(inputs, trace=False, rows=R, ncores=NCORES):
    from concourse.bass_utils import run_bass_kernel_spmd

    full = _prep_inputs(**inputs)
    shared = {k: v for k, v in full.items() if k not in ("slabInit", "thetaT4")}
    in_maps = []
    for c in range(ncores):
        r0 = c * rows
        m = dict(shared)
        m["slabInit"] = np.ascontiguousarray(full["slabInit"][:, r0 : r0 + rows])
        m["thetaT4"] = np.ascontiguousarray(full["thetaT4"][:, r0 : r0 + rows])
        in_maps.append(m)

    nc = _get_nc(rows)
    res = run_bass_kernel_spmd(
        nc, in_maps, core_ids=list(range(ncores)), trace=trace
    )
    out = np.concatenate([res.results[c]["y"] for c in range(ncores)])
    return out, res


def kernel(**inputs):
    out, _ = _run(inputs)
    return out.astype(np.float32)


# revision 35
# speedup vs baseline: 2.0000x; 1.2482x over previous
"""ConditionalRealNVP.log_prob Trainium2 kernel (8-core data parallel), v4.

Contract: kernel(**inputs) takes the FULL inputs from setup_inputs() and
returns the FULL [B] float32 output of reference().

Strategy
--------
Pure data parallel over the batch: B=524288 rows -> 8 cores x 65536 rows,
tiles of 512 rows.  Everything is feature-major; no PE transposes, no
partition-packing: the op COUNT is minimized (every engine op costs
~0.3-0.7us in dispatch+semaphores regardless of size).

  - One resident slab [69, BT] per tile: rows 0-63 h, rows 64-67 the four
    x coords, row 68 ones (bf16); ONE DMA from a host-packed array.
    mm1 contracts K=69 with per-layer W1 (zero rows for unused coords).
  - mm3 full-width: lhsT [128, 4] with live coords in cols TRANS[l],
    zeros elsewhere -> st [4, 512] at partitions 0-3.  Keep-coords get
    s=0/t=0 so the uniform update x' = (s+1)x + t is an exact no-op.
  - x master xpk [4, 512] f32 at partitions 0-3 (one 8KB DMA init);
    updated by TWO fused scalar_tensor_tensor ops (linear exp,
    biases folded):  A = (s_raw + (b3s+1))*x ; x' = (t_raw + b3t) + A.
    Bridge to slab rows 64-67 = ONE GPSIMD cross-quadrant cast-copy.
  - logdet: sum_j s_j = g2s @ (W3s.1), one [1,512] matmul per layer into
    the y PSUM bank; log_pz via one K=4 matmul of -0.5*x^2.  +C and all
    bias constants folded into the final PSUM->SBUF tensor_scalar.
  - Activations: ACT does gelu1s (table Gelu), gelu2s (Square w/
    scale+bias) and the t-net gelu2 affine (Identity w/ scale+bias);
    DVE does the t-net gelu1 (quad, 2 ops); GPSIMD squares a2.
"""

import math

import numpy as np

B = 524288
D = 4
CTX = 64
HID = 128
IN = 69  # 64 h-rows + 4 x rows + ones row
L = 4
KEEP = ((0, 1), (1, 2), (2, 3), (0, 3))
TRANS = ((2, 3), (0, 3), (0, 1), (1, 2))
NCORES = 8
R = B // NCORES  # rows per core
BT = 512  # rows per tile
LOG2PI = 1.8378770664093453
OUT_CONST = -0.5 * D * LOG2PI

# gelu(z) ~= (GA*z + GC)^2 - GC^2  (quadratic gelu)
GA = math.sqrt(1.0 / math.sqrt(2.0 * math.pi))
GC = 0.25 / GA

_CACHE = {}

# layers whose t-net gelu1 runs as table-Gelu on ACT (rest: quad on DVE);
# chosen to balance ACT vs DVE occupancy.  Host folds GA^2 into W2t only
# for the DVE-quad layers.
G1T_ACT_LAYERS = (0, 2)


def _build_nc(rows):
    import concourse.tile as tile
    from concourse import bacc, mybir

    dt = mybir.dt
    F32, BF16 = dt.float32, dt.bfloat16
    AF = mybir.ActivationFunctionType
    OP = mybir.AluOpType

    nt = rows // BT

    nc = bacc.Bacc("TRN2")
    slabInit = nc.dram_tensor("slabInit", [IN, rows], BF16, kind="ExternalInput")
    thetaT4 = nc.dram_tensor("thetaT4", [4, rows], F32, kind="ExternalInput")
    w1 = nc.dram_tensor("w1", [2 * L, IN, HID], BF16, kind="ExternalInput")
    w2 = nc.dram_tensor("w2", [2 * L, HID, HID], BF16, kind="ExternalInput")
    w3 = nc.dram_tensor("w3", [2 * L, HID, 4], BF16, kind="ExternalInput")
    w3sum = nc.dram_tensor("w3sum", [HID, L], BF16, kind="ExternalInput")
    actb_s = nc.dram_tensor("actb_s", [HID, L], F32, kind="ExternalInput")
    actb_t = nc.dram_tensor("actb_t", [HID, L], F32, kind="ExternalInput")
    bsp1 = nc.dram_tensor("bsp1", [4, L], F32, kind="ExternalInput")
    btv = nc.dram_tensor("btv", [4, L], F32, kind="ExternalInput")
    csq = nc.dram_tensor("csq", [4, 1], BF16, kind="ExternalInput")
    yconst = nc.dram_tensor("yconst", [1, 1], F32, kind="ExternalInput")
    y = nc.dram_tensor("y", [rows], F32, kind="ExternalOutput")

    TWO_C1 = 2.0 * GC / GA

    with tile.TileContext(nc) as tc:
        with (
            tc.tile_pool(name="singles", bufs=1) as singles,
            tc.tile_pool(name="slabs", bufs=4) as slabp,
            tc.tile_pool(name="state", bufs=4) as state,
            tc.tile_pool(name="work", bufs=4) as work,
            tc.tile_pool(name="hp", bufs=3, space="PSUM") as hp,
            tc.tile_pool(name="stp", bufs=2, space="PSUM") as stp,
            tc.tile_pool(name="yp", bufs=3, space="PSUM") as yp,
        ):
            # ---- resident constants ----
            w1_sb = singles.tile([IN, 2 * L, HID], BF16)
            nc.sync.dma_start(w1_sb[:], w1[:].rearrange("n k m -> k n m"))
            w2_sb = singles.tile([HID, 2 * L, HID], BF16)
            nc.sync.dma_start(w2_sb[:], w2[:].rearrange("n k m -> k n m"))
            w3_sb = singles.tile([HID, 2 * L, 4], BF16)
            nc.sync.dma_start(w3_sb[:], w3[:].rearrange("n k m -> k n m"))
            w3sum_sb = singles.tile([HID, L], BF16)
            nc.sync.dma_start(w3sum_sb[:], w3sum[:])
            actbs_sb = singles.tile([HID, L], F32)
            nc.sync.dma_start(actbs_sb[:], actb_s[:])
            actbt_sb = singles.tile([HID, L], F32)
            nc.sync.dma_start(actbt_sb[:], actb_t[:])
            bsp1_sb = singles.tile([4, L], F32)
            nc.sync.dma_start(bsp1_sb[:], bsp1[:])
            btv_sb = singles.tile([4, L], F32)
            nc.sync.dma_start(btv_sb[:], btv[:])
            csq_sb = singles.tile([4, 1], BF16)
            nc.sync.dma_start(csq_sb[:], csq[:])
            yconst_sb = singles.tile([1, 1], F32)
            nc.sync.dma_start(yconst_sb[:], yconst[:])

            def tile_gen(it):
                """One tile's instruction stream, yielding between stages so
                the driver can interleave several tiles (software pipelining:
                each engine then has independent work to fill dependency
                stalls)."""
                r0 = it * BT

                slab = slabp.tile([IN, BT], BF16, tag="slab")
                nc.sync.dma_start(slab[:], slabInit[:, r0 : r0 + BT])
                xpk = state.tile([4, BT], F32, tag="xpk")
                nc.sync.dma_start(xpk[:], thetaT4[:, r0 : r0 + BT])
                yps = yp.tile([1, BT], F32, tag="yps")
                yield

                for l in range(L):
                    si, ti = 2 * l, 2 * l + 1

                    h1s = hp.tile([128, BT], F32, tag="h")
                    nc.tensor.matmul(
                        h1s[:], w1_sb[:, si, :], slab[:], start=True, stop=True
                    )
                    h1t = hp.tile([128, BT], F32, tag="h")
                    nc.tensor.matmul(
                        h1t[:], w1_sb[:, ti, :], slab[:], start=True, stop=True
                    )
                    yield

                    # gelu1: s exact table on ACT; t table on ACT for even
                    # layers, quadratic on DVE for odd (engine balance)
                    g1s = work.tile([128, BT], BF16, tag="g1s")
                    nc.scalar.activation(g1s[:], h1s[:], AF.Gelu)
                    g1t = work.tile([128, BT], BF16, tag="g1t")
                    if l in G1T_ACT_LAYERS:
                        nc.scalar.activation(g1t[:], h1t[:], AF.Gelu)
                    else:
                        a1 = work.tile([128, BT], BF16, tag="a1")
                        nc.vector.tensor_scalar(
                            a1[:], h1t[:], TWO_C1, None, OP.add
                        )
                        nc.vector.scalar_tensor_tensor(
                            g1t[:], a1[:], -TWO_C1, a1[:], OP.add, OP.mult
                        )
                    yield

                    h2s = hp.tile([128, BT], F32, tag="h")
                    nc.tensor.matmul(
                        h2s[:], w2_sb[:, si, :], g1s[:], start=True, stop=True
                    )
                    h2t = hp.tile([128, BT], F32, tag="h")
                    nc.tensor.matmul(
                        h2t[:], w2_sb[:, ti, :], g1t[:], start=True, stop=True
                    )
                    yield

                    # gelu2: exact table Gelu with the b2 bias folded in
                    g2s = work.tile([128, BT], BF16, tag="g2s")
                    nc.scalar.activation(
                        g2s[:], h2s[:], AF.Gelu, bias=actbs_sb[:, l : l + 1]
                    )
                    g2t = work.tile([128, BT], BF16, tag="g2t")
                    nc.scalar.activation(
                        g2t[:], h2t[:], AF.Gelu, bias=actbt_sb[:, l : l + 1]
                    )
                    yield

                    # logdet matmul + mm3
                    nc.tensor.matmul(
                        yps[0:1, :], w3sum_sb[:, l : l + 1], g2s[:],
                        start=(l == 0), stop=False, skip_group_check=True,
                    )
                    sts = stp.tile([4, BT], F32, tag="st")
                    nc.tensor.matmul(
                        sts[:], w3_sb[:, si, :], g2s[:], start=True, stop=True
                    )
                    stt = stp.tile([4, BT], F32, tag="st")
                    nc.tensor.matmul(
                        stt[:], w3_sb[:, ti, :], g2t[:], start=True, stop=True
                    )
                    yield

                    # x' = (s_raw + b3s + 1)*x + (t_raw + b3t)
                    av = work.tile([4, BT], F32, tag="av")
                    nc.vector.scalar_tensor_tensor(
                        av[:], sts[:], bsp1_sb[:, l : l + 1], xpk[:],
                        OP.add, OP.mult,
                    )
                    nc.vector.scalar_tensor_tensor(
                        xpk[:], stt[:], btv_sb[:, l : l + 1], av[:],
                        OP.add, OP.add,
                    )
                    if l < 3:
                        nc.gpsimd.tensor_copy(slab[64:68, :], xpk[:])
                    yield

                # tail: yps += -0.5 * x^2; y = yps + const
                sq = work.tile([4, BT], BF16, tag="sq")
                nc.gpsimd.tensor_mul(sq[:], xpk[:], xpk[:])
                yield
                nc.tensor.matmul(
                    yps[0:1, :], csq_sb[:], sq[:],
                    start=False, stop=True, skip_group_check=True,
                )
                ysb = work.tile([1, BT], F32, tag="ysb")
                nc.vector.tensor_scalar(
                    ysb[:], yps[:], yconst_sb[0:1, 0:1], None, OP.add
                )
                nc.sync.dma_start(
                    y[r0 : r0 + BT].rearrange("(a b) -> a b", a=1), ysb[:]
                )

            # rolling-window software pipeline over tiles
            from collections import deque

            W = 3
            active = deque()
            next_it = 0
            while active or next_it < nt:
                while len(active) < W and next_it < nt:
                    active.append(tile_gen(next_it))
                    next_it += 1
                g = active.popleft()
                try:
                    next(g)
                    active.append(g)
                except StopIteration:
                    pass

    nc.compile()
    return nc


def _prep_inputs(theta, h, sW1, sb1, sW2, sb2, sW3, sb3, tW1, tb1, tW2, tb2, tW3, tb3):
    """Host-side packing/folding. Returns dict of full-size arrays."""
    import ml_dtypes

    bf16 = ml_dtypes.bfloat16
    f32 = np.float32
    theta = np.asarray(theta, f32)
    h = np.asarray(h, f32)

    # slab rows: 0-63 h.T, 64-67 theta.T, 68 ones
    slabInit = np.empty((IN, B), bf16)
    slabInit[0:64, :] = np.ascontiguousarray(h.T).astype(bf16)
    slabInit[64:68, :] = np.ascontiguousarray(theta.T).astype(bf16)
    slabInit[68, :] = np.ones((B,), bf16)
    thetaT4 = np.ascontiguousarray(theta.T).astype(f32)

    w1 = np.zeros((2 * L, IN, HID), f32)
    w2 = np.zeros((2 * L, HID, HID), f32)
    w3 = np.zeros((2 * L, HID, 4), f32)
    w3sum = np.zeros((HID, L), f32)
    actb_s = np.zeros((HID, L), f32)
    actb_t = np.zeros((HID, L), f32)
    bsp1 = np.ones((4, L), f32)
    btv = np.zeros((4, L), f32)
    yconst = OUT_CONST
    for i in range(L):
        t0, t1 = TRANS[i]
        for j, (W1, B1, W2_, B2, W3_, B3) in enumerate(
            ((sW1, sb1, sW2, sb2, sW3, sb3), (tW1, tb1, tW2, tb2, tW3, tb3))
        ):
            n = 2 * i + j
            W1i, B1i = np.asarray(W1[i], f32), np.asarray(B1[i], f32)
            W2i, B2i = np.asarray(W2_[i], f32), np.asarray(B2[i], f32)
            W3i, B3i = np.asarray(W3_[i], f32), np.asarray(B3[i], f32)
            # mm1 rows: [h(64); x0..x3 (keep coords only); b1]
            w1[n, 0:64, :] = W1i[2:66]
            k0, k1 = KEEP[i]
            w1[n, 64 + k0, :] = W1i[0]
            w1[n, 64 + k1, :] = W1i[1]
            w1[n, 68, :] = B1i
            # mm2: GA^2 folded into W2t only for DVE-quad-gelu1 layers
            if j == 0 or i in G1T_ACT_LAYERS:
                w2[n] = W2i
            else:
                w2[n] = (GA * GA) * W2i
            # gelu2 is exact table Gelu with bias = b2 (scale 1)
            if j == 0:
                actb_s[:, i] = B2i
            else:
                actb_t[:, i] = B2i
            # mm3: [128, 4], live coords in cols t0/t1
            w3[n, :, t0] = W3i[:, 0]
            w3[n, :, t1] = W3i[:, 1]
            b3eff = B3i
            if j == 0:
                # logdet pieces: w3sum + bias constants into yconst
                w3sum[:, i] = W3i[:, 0] + W3i[:, 1]
                yconst += b3eff.sum()
                bsp1[t0, i] = b3eff[0] + 1.0
                bsp1[t1, i] = b3eff[1] + 1.0
            else:
                btv[t0, i] = b3eff[0]
                btv[t1, i] = b3eff[1]

    csq = np.full((4, 1), -0.5, f32)

    return {
        "slabInit": slabInit,
        "thetaT4": thetaT4,
        "w1": w1.astype(bf16),
        "w2": w2.astype(bf16),
        "w3": w3.astype(bf16),
        "w3sum": w3sum.astype(bf16),
        "actb_s": actb_s,
        "actb_t": actb_t,
        "bsp1": bsp1,
        "btv": btv,
        "csq": csq.astype(bf16),
        "yconst": np.full((1, 1), yconst, f32),
    }


def _get_nc(rows):
    key = ("nc", rows)
    if key not in _CACHE:
        _CACHE[key] = _build_nc(rows)
    return _CACHE[key]


def _run(inputs, trace=False, rows=R, ncores=NCORES):
    from concourse.bass_utils import run_bass_kernel_spmd

    full = _prep_inputs(**inputs)
    shared = {k: v for k, v in full.items() if k not in ("slabInit", "thetaT4")}
    in_maps = []
    for c in range(ncores):
        r0 = c * rows
        m = dict(shared)
        m["slabInit"] = np.ascontiguousarray(full["slabInit"][:, r0 : r0 + rows])
        m["thetaT4"] = np.ascontiguousarray(full["thetaT4"][:, r0 : r0 + rows])
        in_maps.append(m)

    nc = _get_nc(rows)
    res = run_bass_kernel_spmd(
        nc, in_maps, core_ids=list(range(ncores)), trace=trace
    )
    out = np.concatenate([res.results[c]["y"] for c in range(ncores)])
    return out, res


def kernel(**inputs):
    out, _ = _run(inputs)
    return out.astype(np.float32)


# revision 36
# speedup vs baseline: 2.1274x; 1.0637x over previous
"""ConditionalRealNVP.log_prob Trainium2 kernel (8-core data parallel), v4.

Contract: kernel(**inputs) takes the FULL inputs from setup_inputs() and
returns the FULL [B] float32 output of reference().

Strategy
--------
Pure data parallel over the batch: B=524288 rows -> 8 cores x 65536 rows,
tiles of 512 rows.  Everything is feature-major; no PE transposes, no
partition-packing: the op COUNT is minimized (every engine op costs
~0.3-0.7us in dispatch+semaphores regardless of size).

  - One resident slab [69, BT] per tile: rows 0-63 h, rows 64-67 the four
    x coords, row 68 ones (bf16); ONE DMA from a host-packed array.
    mm1 contracts K=69 with per-layer W1 (zero rows for unused coords).
  - mm3 full-width: lhsT [128, 4] with live coords in cols TRANS[l],
    zeros elsewhere -> st [4, 512] at partitions 0-3.  Keep-coords get
    s=0/t=0 so the uniform update x' = (s+1)x + t is an exact no-op.
  - x master xpk [4, 512] f32 at partitions 0-3 (one 8KB DMA init);
    updated by TWO fused scalar_tensor_tensor ops (linear exp,
    biases folded):  A = (s_raw + (b3s+1))*x ; x' = (t_raw + b3t) + A.
    Bridge to slab rows 64-67 = ONE GPSIMD cross-quadrant cast-copy.
  - logdet: sum_j s_j = g2s @ (W3s.1), one [1,512] matmul per layer into
    the y PSUM bank; log_pz via one K=4 matmul of -0.5*x^2.  +C and all
    bias constants folded into the final PSUM->SBUF tensor_scalar.
  - Activations: ACT does gelu1s (table Gelu), gelu2s (Square w/
    scale+bias) and the t-net gelu2 affine (Identity w/ scale+bias);
    DVE does the t-net gelu1 (quad, 2 ops); GPSIMD squares a2.
"""

import math

import numpy as np

B = 524288
D = 4
CTX = 64
HID = 128
IN = 69  # 64 h-rows + 4 x rows + ones row
L = 4
KEEP = ((0, 1), (1, 2), (2, 3), (0, 3))
TRANS = ((2, 3), (0, 3), (0, 1), (1, 2))
NCORES = 8
R = B // NCORES  # rows per core
BT = 512  # rows per tile
LOG2PI = 1.8378770664093453
OUT_CONST = -0.5 * D * LOG2PI

# gelu(z) ~= (GA*z + GC)^2 - GC^2  (quadratic gelu)
GA = math.sqrt(1.0 / math.sqrt(2.0 * math.pi))
GC = 0.25 / GA

_CACHE = {}

# layers whose t-net gelu1 runs as table-Gelu on ACT (rest: quad on DVE);
# chosen to balance ACT vs DVE occupancy.  Host folds GA^2 into W2t only
# for the DVE-quad layers.
G1T_ACT_LAYERS = (0, 2)


def _build_nc(rows):
    import concourse.tile as tile
    from concourse import bacc, mybir

    dt = mybir.dt
    F32, BF16 = dt.float32, dt.bfloat16
    AF = mybir.ActivationFunctionType
    OP = mybir.AluOpType

    nt = rows // BT

    nc = bacc.Bacc("TRN2")
    slabInit = nc.dram_tensor("slabInit", [IN, rows], BF16, kind="ExternalInput")
    thetaT4 = nc.dram_tensor("thetaT4", [4, rows], F32, kind="ExternalInput")
    w1 = nc.dram_tensor("w1", [2 * L, IN, HID], BF16, kind="ExternalInput")
    w2 = nc.dram_tensor("w2", [2 * L, HID, HID], BF16, kind="ExternalInput")
    w3 = nc.dram_tensor("w3", [2 * L, HID, 4], BF16, kind="ExternalInput")
    w3sum = nc.dram_tensor("w3sum", [HID, L], BF16, kind="ExternalInput")
    actb_s = nc.dram_tensor("actb_s", [HID, L], F32, kind="ExternalInput")
    actb_t = nc.dram_tensor("actb_t", [HID, L], F32, kind="ExternalInput")
    bsp1 = nc.dram_tensor("bsp1", [4, L], F32, kind="ExternalInput")
    btv = nc.dram_tensor("btv", [4, L], F32, kind="ExternalInput")
    csq = nc.dram_tensor("csq", [4, 1], BF16, kind="ExternalInput")
    yconst = nc.dram_tensor("yconst", [1, 1], F32, kind="ExternalInput")
    y = nc.dram_tensor("y", [rows], F32, kind="ExternalOutput")

    TWO_C1 = 2.0 * GC / GA

    with tile.TileContext(nc) as tc:
        with (
            tc.tile_pool(name="singles", bufs=1) as singles,
            tc.tile_pool(name="slabs", bufs=4) as slabp,
            tc.tile_pool(name="state", bufs=4) as state,
            tc.tile_pool(name="work", bufs=4) as work,
            tc.tile_pool(name="hp", bufs=3, space="PSUM") as hp,
            tc.tile_pool(name="stp", bufs=2, space="PSUM") as stp,
            tc.tile_pool(name="yp", bufs=3, space="PSUM") as yp,
        ):
            # ---- resident constants ----
            w1_sb = singles.tile([IN, 2 * L, HID], BF16)
            nc.sync.dma_start(w1_sb[:], w1[:].rearrange("n k m -> k n m"))
            w2_sb = singles.tile([HID, 2 * L, HID], BF16)
            nc.sync.dma_start(w2_sb[:], w2[:].rearrange("n k m -> k n m"))
            w3_sb = singles.tile([HID, 2 * L, 4], BF16)
            nc.sync.dma_start(w3_sb[:], w3[:].rearrange("n k m -> k n m"))
            w3sum_sb = singles.tile([HID, L], BF16)
            nc.sync.dma_start(w3sum_sb[:], w3sum[:])
            actbs_sb = singles.tile([HID, L], F32)
            nc.sync.dma_start(actbs_sb[:], actb_s[:])
            actbt_sb = singles.tile([HID, L], F32)
            nc.sync.dma_start(actbt_sb[:], actb_t[:])
            bsp1_sb = singles.tile([4, L], F32)
            nc.sync.dma_start(bsp1_sb[:], bsp1[:])
            btv_sb = singles.tile([4, L], F32)
            nc.sync.dma_start(btv_sb[:], btv[:])
            csq_sb = singles.tile([4, 1], BF16)
            nc.sync.dma_start(csq_sb[:], csq[:])
            yconst_sb = singles.tile([1, 1], F32)
            nc.sync.dma_start(yconst_sb[:], yconst[:])

            def tile_gen(it):
                """One tile's instruction stream, yielding between stages so
                the driver can interleave several tiles (software pipelining:
                each engine then has independent work to fill dependency
                stalls)."""
                r0 = it * BT

                slab = slabp.tile([IN, BT], BF16, tag="slab")
                nc.sync.dma_start(slab[:], slabInit[:, r0 : r0 + BT])
                xpk = state.tile([4, BT], F32, tag="xpk")
                nc.sync.dma_start(xpk[:], thetaT4[:, r0 : r0 + BT])
                yps = yp.tile([1, BT], F32, tag="yps")
                yield

                for l in range(L):
                    si, ti = 2 * l, 2 * l + 1

                    h1s = hp.tile([128, BT], F32, tag="h")
                    nc.tensor.matmul(
                        h1s[:], w1_sb[:, si, :], slab[:], start=True, stop=True
                    )
                    h1t = hp.tile([128, BT], F32, tag="h")
                    nc.tensor.matmul(
                        h1t[:], w1_sb[:, ti, :], slab[:], start=True, stop=True
                    )
                    yield

                    # gelu1: s exact table on ACT; t table on ACT for even
                    # layers, quadratic on DVE for odd (engine balance)
                    g1s = work.tile([128, BT], BF16, tag="g1s")
                    nc.scalar.activation(g1s[:], h1s[:], AF.Gelu)
                    g1t = work.tile([128, BT], BF16, tag="g1t")
                    if l in G1T_ACT_LAYERS:
                        nc.scalar.activation(g1t[:], h1t[:], AF.Gelu)
                    else:
                        a1 = work.tile([128, BT], BF16, tag="a1")
                        nc.vector.tensor_scalar(
                            a1[:], h1t[:], TWO_C1, None, OP.add
                        )
                        nc.vector.scalar_tensor_tensor(
                            g1t[:], a1[:], -TWO_C1, a1[:], OP.add, OP.mult
                        )
                    yield

                    h2s = hp.tile([128, BT], F32, tag="h")
                    nc.tensor.matmul(
                        h2s[:], w2_sb[:, si, :], g1s[:], start=True, stop=True
                    )
                    h2t = hp.tile([128, BT], F32, tag="h")
                    nc.tensor.matmul(
                        h2t[:], w2_sb[:, ti, :], g1t[:], start=True, stop=True
                    )
                    yield

                    # gelu2: exact table Gelu with the b2 bias folded in
                    g2s = work.tile([128, BT], BF16, tag="g2s")
                    nc.scalar.activation(
                        g2s[:], h2s[:], AF.Gelu, bias=actbs_sb[:, l : l + 1]
                    )
                    g2t = work.tile([128, BT], BF16, tag="g2t")
                    nc.scalar.activation(
                        g2t[:], h2t[:], AF.Gelu, bias=actbt_sb[:, l : l + 1]
                    )
                    yield

                    # logdet matmul + mm3
                    nc.tensor.matmul(
                        yps[0:1, :], w3sum_sb[:, l : l + 1], g2s[:],
                        start=(l == 0), stop=False, skip_group_check=True,
                    )
                    sts = stp.tile([4, BT], F32, tag="st")
                    nc.tensor.matmul(
                        sts[:], w3_sb[:, si, :], g2s[:], start=True, stop=True
                    )
                    stt = stp.tile([4, BT], F32, tag="st")
                    nc.tensor.matmul(
                        stt[:], w3_sb[:, ti, :], g2t[:], start=True, stop=True
                    )
                    yield

                    # x' = (s_raw + b3s + 1)*x + (t_raw + b3t)
                    av = work.tile([4, BT], F32, tag="av")
                    nc.vector.scalar_tensor_tensor(
                        av[:], sts[:], bsp1_sb[:, l : l + 1], xpk[:],
                        OP.add, OP.mult,
                    )
                    nc.vector.scalar_tensor_tensor(
                        xpk[:], stt[:], btv_sb[:, l : l + 1], av[:],
                        OP.add, OP.add,
                    )
                    if l < 3:
                        nc.gpsimd.tensor_copy(
                            slab[64:68, 0:256], xpk[:, 0:256]
                        )
                        nc.vector.tensor_copy(
                            slab[64:68, 256:512], xpk[:, 256:512]
                        )
                    yield

                # tail: yps += -0.5 * x^2; y = yps + const
                sq = work.tile([4, BT], BF16, tag="sq")
                nc.gpsimd.tensor_mul(sq[:], xpk[:], xpk[:])
                yield
                nc.tensor.matmul(
                    yps[0:1, :], csq_sb[:], sq[:],
                    start=False, stop=True, skip_group_check=True,
                )
                ysb = work.tile([1, BT], F32, tag="ysb")
                nc.vector.tensor_scalar(
                    ysb[:], yps[:], yconst_sb[0:1, 0:1], None, OP.add
                )
                nc.sync.dma_start(
                    y[r0 : r0 + BT].rearrange("(a b) -> a b", a=1), ysb[:]
                )

            # rolling-window software pipeline over tiles
            from collections import deque

            W = 3
            active = deque()
            next_it = 0
            while active or next_it < nt:
                while len(active) < W and next_it < nt:
                    active.append(tile_gen(next_it))
                    next_it += 1
                g = active.popleft()
                try:
                    next(g)
                    active.append(g)
                except StopIteration:
                    pass

    nc.compile()
    return nc


def _prep_inputs(theta, h, sW1, sb1, sW2, sb2, sW3, sb3, tW1, tb1, tW2, tb2, tW3, tb3):
    """Host-side packing/folding. Returns dict of full-size arrays."""
    import ml_dtypes

    bf16 = ml_dtypes.bfloat16
    f32 = np.float32
    theta = np.asarray(theta, f32)
    h = np.asarray(h, f32)

    # slab rows: 0-63 h.T, 64-67 theta.T, 68 ones
    slabInit = np.empty((IN, B), bf16)
    slabInit[0:64, :] = np.ascontiguousarray(h.T).astype(bf16)
    slabInit[64:68, :] = np.ascontiguousarray(theta.T).astype(bf16)
    slabInit[68, :] = np.ones((B,), bf16)
    thetaT4 = np.ascontiguousarray(theta.T).astype(f32)

    w1 = np.zeros((2 * L, IN, HID), f32)
    w2 = np.zeros((2 * L, HID, HID), f32)
    w3 = np.zeros((2 * L, HID, 4), f32)
    w3sum = np.zeros((HID, L), f32)
    actb_s = np.zeros((HID, L), f32)
    actb_t = np.zeros((HID, L), f32)
    bsp1 = np.ones((4, L), f32)
    btv = np.zeros((4, L), f32)
    yconst = OUT_CONST
    for i in range(L):
        t0, t1 = TRANS[i]
        for j, (W1, B1, W2_, B2, W3_, B3) in enumerate(
            ((sW1, sb1, sW2, sb2, sW3, sb3), (tW1, tb1, tW2, tb2, tW3, tb3))
        ):
            n = 2 * i + j
            W1i, B1i = np.asarray(W1[i], f32), np.asarray(B1[i], f32)
            W2i, B2i = np.asarray(W2_[i], f32), np.asarray(B2[i], f32)
            W3i, B3i = np.asarray(W3_[i], f32), np.asarray(B3[i], f32)
            # mm1 rows: [h(64); x0..x3 (keep coords only); b1]
            w1[n, 0:64, :] = W1i[2:66]
            k0, k1 = KEEP[i]
            w1[n, 64 + k0, :] = W1i[0]
            w1[n, 64 + k1, :] = W1i[1]
            w1[n, 68, :] = B1i
            # mm2: GA^2 folded into W2t only for DVE-quad-gelu1 layers
            if j == 0 or i in G1T_ACT_LAYERS:
                w2[n] = W2i
            else:
                w2[n] = (GA * GA) * W2i
            # gelu2 is exact table Gelu with bias = b2 (scale 1)
            if j == 0:
                actb_s[:, i] = B2i
            else:
                actb_t[:, i] = B2i
            # mm3: [128, 4], live coords in cols t0/t1
            w3[n, :, t0] = W3i[:, 0]
            w3[n, :, t1] = W3i[:, 1]
            b3eff = B3i
            if j == 0:
                # logdet pieces: w3sum + bias constants into yconst
                w3sum[:, i] = W3i[:, 0] + W3i[:, 1]
                yconst += b3eff.sum()
                bsp1[t0, i] = b3eff[0] + 1.0
                bsp1[t1, i] = b3eff[1] + 1.0
            else:
                btv[t0, i] = b3eff[0]
                btv[t1, i] = b3eff[1]

    csq = np.full((4, 1), -0.5, f32)

    return {
        "slabInit": slabInit,
        "thetaT4": thetaT4,
        "w1": w1.astype(bf16),
        "w2": w2.astype(bf16),
        "w3": w3.astype(bf16),
        "w3sum": w3sum.astype(bf16),
        "actb_s": actb_s,
        "actb_t": actb_t,
        "bsp1": bsp1,
        "btv": btv,
        "csq": csq.astype(bf16),
        "yconst": np.full((1, 1), yconst, f32),
    }


def _get_nc(rows):
    key = ("nc", rows)
    if key not in _CACHE:
        _CACHE[key] = _build_nc(rows)
    return _CACHE[key]


def _run(inputs, trace=False, rows=R, ncores=NCORES):
    from concourse.bass_utils import run_bass_kernel_spmd

    full = _prep_inputs(**inputs)
    shared = {k: v for k, v in full.items() if k not in ("slabInit", "thetaT4")}
    in_maps = []
    for c in range(ncores):
        r0 = c * rows
        m = dict(shared)
        m["slabInit"] = np.ascontiguousarray(full["slabInit"][:, r0 : r0 + rows])
        m["thetaT4"] = np.ascontiguousarray(full["thetaT4"][:, r0 : r0 + rows])
        in_maps.append(m)

    nc = _get_nc(rows)
    res = run_bass_kernel_spmd(
        nc, in_maps, core_ids=list(range(ncores)), trace=trace
    )
    out = np.concatenate([res.results[c]["y"] for c in range(ncores)])
    return out, res


def kernel(**inputs):
    out, _ = _run(inputs)
    return out.astype(np.float32)


# revision 39
# speedup vs baseline: 2.2336x; 1.0499x over previous
"""ConditionalRealNVP.log_prob Trainium2 kernel (8-core data parallel), v4.

Contract: kernel(**inputs) takes the FULL inputs from setup_inputs() and
returns the FULL [B] float32 output of reference().

Strategy
--------
Pure data parallel over the batch: B=524288 rows -> 8 cores x 65536 rows,
tiles of 512 rows.  Everything is feature-major; no PE transposes, no
partition-packing: the op COUNT is minimized (every engine op costs
~0.3-0.7us in dispatch+semaphores regardless of size).

  - One resident slab [69, BT] per tile: rows 0-63 h, rows 64-67 the four
    x coords, row 68 ones (bf16); ONE DMA from a host-packed array.
    mm1 contracts K=69 with per-layer W1 (zero rows for unused coords).
  - mm3 full-width: lhsT [128, 4] with live coords in cols TRANS[l],
    zeros elsewhere -> st [4, 512] at partitions 0-3.  Keep-coords get
    s=0/t=0 so the uniform update x' = (s+1)x + t is an exact no-op.
  - x master xpk [4, 512] f32 at partitions 0-3 (one 8KB DMA init);
    updated by TWO fused scalar_tensor_tensor ops (linear exp,
    biases folded):  A = (s_raw + (b3s+1))*x ; x' = (t_raw + b3t) + A.
    Bridge to slab rows 64-67 = ONE GPSIMD cross-quadrant cast-copy.
  - logdet: sum_j s_j = g2s @ (W3s.1), one [1,512] matmul per layer into
    the y PSUM bank; log_pz via one K=4 matmul of -0.5*x^2.  +C and all
    bias constants folded into the final PSUM->SBUF tensor_scalar.
  - Activations: ACT does gelu1s (table Gelu), gelu2s (Square w/
    scale+bias) and the t-net gelu2 affine (Identity w/ scale+bias);
    DVE does the t-net gelu1 (quad, 2 ops); GPSIMD squares a2.
"""

import math

import numpy as np

B = 524288
D = 4
CTX = 64
HID = 128
IN = 69  # 64 h-rows + 4 x rows + ones row
L = 4
KEEP = ((0, 1), (1, 2), (2, 3), (0, 3))
TRANS = ((2, 3), (0, 3), (0, 1), (1, 2))
NCORES = 8
R = B // NCORES  # rows per core
BT = 512  # rows per tile
LOG2PI = 1.8378770664093453
OUT_CONST = -0.5 * D * LOG2PI

# gelu(z) ~= (GA*z + GC)^2 - GC^2  (quadratic gelu)
GA = math.sqrt(1.0 / math.sqrt(2.0 * math.pi))
GC = 0.25 / GA

_CACHE = {}

# layers whose t-net gelu1 runs as table-Gelu on ACT (rest: quad on DVE);
# chosen to balance ACT vs DVE occupancy.  Host folds GA^2 into W2t only
# for the DVE-quad layers.
G1T_ACT_LAYERS = (0, 2)


def _build_nc(rows):
    import concourse.tile as tile
    from concourse import bacc, mybir

    dt = mybir.dt
    F32, BF16 = dt.float32, dt.bfloat16
    AF = mybir.ActivationFunctionType
    OP = mybir.AluOpType

    nt = rows // BT

    nc = bacc.Bacc("TRN2")
    slabInit = nc.dram_tensor("slabInit", [IN, rows], BF16, kind="ExternalInput")
    thetaT4 = nc.dram_tensor("thetaT4", [4, rows], F32, kind="ExternalInput")
    w1 = nc.dram_tensor("w1", [2 * L, IN, HID], BF16, kind="ExternalInput")
    w2 = nc.dram_tensor("w2", [2 * L, HID, HID], BF16, kind="ExternalInput")
    w3 = nc.dram_tensor("w3", [2 * L, HID, 4], BF16, kind="ExternalInput")
    w3sum = nc.dram_tensor("w3sum", [HID, L], BF16, kind="ExternalInput")
    actb_s = nc.dram_tensor("actb_s", [HID, L], F32, kind="ExternalInput")
    actb_t = nc.dram_tensor("actb_t", [HID, L], F32, kind="ExternalInput")
    bsp1 = nc.dram_tensor("bsp1", [4, L], F32, kind="ExternalInput")
    btv = nc.dram_tensor("btv", [4, L], F32, kind="ExternalInput")
    csq = nc.dram_tensor("csq", [4, 1], BF16, kind="ExternalInput")
    yconst = nc.dram_tensor("yconst", [1, 1], F32, kind="ExternalInput")
    y = nc.dram_tensor("y", [rows], F32, kind="ExternalOutput")

    TWO_C1 = 2.0 * GC / GA

    with tile.TileContext(nc) as tc:
        with (
            tc.tile_pool(name="singles", bufs=1) as singles,
            tc.tile_pool(name="slabs", bufs=4) as slabp,
            tc.tile_pool(name="state", bufs=4) as state,
            tc.tile_pool(name="work", bufs=4) as work,
            tc.tile_pool(name="hp", bufs=3, space="PSUM") as hp,
            tc.tile_pool(name="stp", bufs=2, space="PSUM") as stp,
            tc.tile_pool(name="yp", bufs=3, space="PSUM") as yp,
        ):
            # ---- resident constants ----
            w1_sb = singles.tile([IN, 2 * L, HID], BF16)
            nc.sync.dma_start(w1_sb[:], w1[:].rearrange("n k m -> k n m"))
            w2_sb = singles.tile([HID, 2 * L, HID], BF16)
            nc.sync.dma_start(w2_sb[:], w2[:].rearrange("n k m -> k n m"))
            w3_sb = singles.tile([HID, 2 * L, 4], BF16)
            nc.sync.dma_start(w3_sb[:], w3[:].rearrange("n k m -> k n m"))
            w3sum_sb = singles.tile([HID, L], BF16)
            nc.sync.dma_start(w3sum_sb[:], w3sum[:])
            actbs_sb = singles.tile([HID, L], F32)
            nc.sync.dma_start(actbs_sb[:], actb_s[:])
            actbt_sb = singles.tile([HID, L], F32)
            nc.sync.dma_start(actbt_sb[:], actb_t[:])
            bsp1_sb = singles.tile([4, L], F32)
            nc.sync.dma_start(bsp1_sb[:], bsp1[:])
            btv_sb = singles.tile([4, L], F32)
            nc.sync.dma_start(btv_sb[:], btv[:])
            csq_sb = singles.tile([4, 1], BF16)
            nc.sync.dma_start(csq_sb[:], csq[:])
            yconst_sb = singles.tile([1, 1], F32)
            nc.sync.dma_start(yconst_sb[:], yconst[:])

            def tile_gen(it):
                """One tile's instruction stream, yielding between stages so
                the driver can interleave several tiles (software pipelining:
                each engine then has independent work to fill dependency
                stalls)."""
                r0 = it * BT

                slab = slabp.tile([IN, BT], BF16, tag="slab")
                nc.sync.dma_start(slab[:], slabInit[:, r0 : r0 + BT])
                xpk = state.tile([4, BT], F32, tag="xpk")
                nc.sync.dma_start(xpk[:], thetaT4[:, r0 : r0 + BT])
                yps = None
                yield

                for l in range(L):
                    si, ti = 2 * l, 2 * l + 1

                    h1s = hp.tile([128, BT], F32, tag="h")
                    nc.tensor.matmul(
                        h1s[:], w1_sb[:, si, :], slab[:], start=True, stop=True
                    )
                    h1t = hp.tile([128, BT], F32, tag="h")
                    nc.tensor.matmul(
                        h1t[:], w1_sb[:, ti, :], slab[:], start=True, stop=True
                    )
                    yield

                    # gelu1: s exact table on ACT; t table on ACT for even
                    # layers, quadratic on DVE for odd (engine balance)
                    g1s = work.tile([128, BT], BF16, tag="g1s")
                    nc.scalar.activation(g1s[:], h1s[:], AF.Gelu)
                    g1t = work.tile([128, BT], BF16, tag="g1t")
                    if l in G1T_ACT_LAYERS:
                        nc.scalar.activation(g1t[:], h1t[:], AF.Gelu)
                    else:
                        a1 = work.tile([128, BT], BF16, tag="a1")
                        nc.vector.tensor_scalar(
                            a1[:], h1t[:], TWO_C1, None, OP.add
                        )
                        nc.vector.scalar_tensor_tensor(
                            g1t[:], a1[:], -TWO_C1, a1[:], OP.add, OP.mult
                        )
                    yield

                    h2s = hp.tile([128, BT], F32, tag="h")
                    nc.tensor.matmul(
                        h2s[:], w2_sb[:, si, :], g1s[:], start=True, stop=True
                    )
                    h2t = hp.tile([128, BT], F32, tag="h")
                    nc.tensor.matmul(
                        h2t[:], w2_sb[:, ti, :], g1t[:], start=True, stop=True
                    )
                    yield

                    # gelu2: exact table Gelu with the b2 bias folded in
                    g2s = work.tile([128, BT], BF16, tag="g2s")
                    nc.scalar.activation(
                        g2s[:], h2s[:], AF.Gelu, bias=actbs_sb[:, l : l + 1]
                    )
                    g2t = work.tile([128, BT], BF16, tag="g2t")
                    nc.scalar.activation(
                        g2t[:], h2t[:], AF.Gelu, bias=actbt_sb[:, l : l + 1]
                    )
                    yield

                    # logdet matmul + mm3
                    if yps is None:
                        yps = yp.tile([1, BT], F32, tag="yps")
                    nc.tensor.matmul(
                        yps[0:1, :], w3sum_sb[:, l : l + 1], g2s[:],
                        start=(l == 0), stop=False, skip_group_check=True,
                    )
                    sts = stp.tile([4, BT], F32, tag="st")
                    nc.tensor.matmul(
                        sts[:], w3_sb[:, si, :], g2s[:], start=True, stop=True
                    )
                    stt = stp.tile([4, BT], F32, tag="st")
                    nc.tensor.matmul(
                        stt[:], w3_sb[:, ti, :], g2t[:], start=True, stop=True
                    )
                    yield

                    # x' = (s_raw + b3s + 1)*x + (t_raw + b3t)
                    av = work.tile([4, BT], F32, tag="av")
                    nc.vector.scalar_tensor_tensor(
                        av[:], sts[:], bsp1_sb[:, l : l + 1], xpk[:],
                        OP.add, OP.mult,
                    )
                    nc.vector.scalar_tensor_tensor(
                        xpk[:], stt[:], btv_sb[:, l : l + 1], av[:],
                        OP.add, OP.add,
                    )
                    if l < 3:
                        nc.gpsimd.tensor_copy(
                            slab[64:68, 0:256], xpk[:, 0:256]
                        )
                        nc.vector.tensor_copy(
                            slab[64:68, 256:512], xpk[:, 256:512]
                        )
                    yield

                # tail: yps += -0.5 * x^2; y = yps + const
                sq = work.tile([4, BT], BF16, tag="sq")
                nc.gpsimd.tensor_mul(sq[:], xpk[:], xpk[:])
                yield
                nc.tensor.matmul(
                    yps[0:1, :], csq_sb[:], sq[:],
                    start=False, stop=True, skip_group_check=True,
                )
                ysb = work.tile([1, BT], F32, tag="ysb")
                nc.vector.tensor_scalar(
                    ysb[:], yps[:], yconst_sb[0:1, 0:1], None, OP.add
                )
                nc.sync.dma_start(
                    y[r0 : r0 + BT].rearrange("(a b) -> a b", a=1), ysb[:]
                )

            # rolling-window software pipeline over tiles
            from collections import deque

            W = 4
            active = deque()
            next_it = 0
            while active or next_it < nt:
                while len(active) < W and next_it < nt:
                    active.append(tile_gen(next_it))
                    next_it += 1
                g = active.popleft()
                try:
                    next(g)
                    active.append(g)
                except StopIteration:
                    pass

    nc.compile()
    return nc


def _prep_inputs(theta, h, sW1, sb1, sW2, sb2, sW3, sb3, tW1, tb1, tW2, tb2, tW3, tb3):
    """Host-side packing/folding. Returns dict of full-size arrays."""
    import ml_dtypes

    bf16 = ml_dtypes.bfloat16
    f32 = np.float32
    theta = np.asarray(theta, f32)
    h = np.asarray(h, f32)

    # slab rows: 0-63 h.T, 64-67 theta.T, 68 ones
    slabInit = np.empty((IN, B), bf16)
    slabInit[0:64, :] = np.ascontiguousarray(h.T).astype(bf16)
    slabInit[64:68, :] = np.ascontiguousarray(theta.T).astype(bf16)
    slabInit[68, :] = np.ones((B,), bf16)
    thetaT4 = np.ascontiguousarray(theta.T).astype(f32)

    w1 = np.zeros((2 * L, IN, HID), f32)
    w2 = np.zeros((2 * L, HID, HID), f32)
    w3 = np.zeros((2 * L, HID, 4), f32)
    w3sum = np.zeros((HID, L), f32)
    actb_s = np.zeros((HID, L), f32)
    actb_t = np.zeros((HID, L), f32)
    bsp1 = np.ones((4, L), f32)
    btv = np.zeros((4, L), f32)
    yconst = OUT_CONST
    for i in range(L):
        t0, t1 = TRANS[i]
        for j, (W1, B1, W2_, B2, W3_, B3) in enumerate(
            ((sW1, sb1, sW2, sb2, sW3, sb3), (tW1, tb1, tW2, tb2, tW3, tb3))
        ):
            n = 2 * i + j
            W1i, B1i = np.asarray(W1[i], f32), np.asarray(B1[i], f32)
            W2i, B2i = np.asarray(W2_[i], f32), np.asarray(B2[i], f32)
            W3i, B3i = np.asarray(W3_[i], f32), np.asarray(B3[i], f32)
            # mm1 rows: [h(64); x0..x3 (keep coords only); b1]
            w1[n, 0:64, :] = W1i[2:66]
            k0, k1 = KEEP[i]
            w1[n, 64 + k0, :] = W1i[0]
            w1[n, 64 + k1, :] = W1i[1]
            w1[n, 68, :] = B1i
            # mm2: GA^2 folded into W2t only for DVE-quad-gelu1 layers
            if j == 0 or i in G1T_ACT_LAYERS:
                w2[n] = W2i
            else:
                w2[n] = (GA * GA) * W2i
            # gelu2 is exact table Gelu with bias = b2 (scale 1)
            if j == 0:
                actb_s[:, i] = B2i
            else:
                actb_t[:, i] = B2i
            # mm3: [128, 4], live coords in cols t0/t1
            w3[n, :, t0] = W3i[:, 0]
            w3[n, :, t1] = W3i[:, 1]
            b3eff = B3i
            if j == 0:
                # logdet pieces: w3sum + bias constants into yconst
                w3sum[:, i] = W3i[:, 0] + W3i[:, 1]
                yconst += b3eff.sum()
                bsp1[t0, i] = b3eff[0] + 1.0
                bsp1[t1, i] = b3eff[1] + 1.0
            else:
                btv[t0, i] = b3eff[0]
                btv[t1, i] = b3eff[1]

    csq = np.full((4, 1), -0.5, f32)

    return {
        "slabInit": slabInit,
        "thetaT4": thetaT4,
        "w1": w1.astype(bf16),
        "w2": w2.astype(bf16),
        "w3": w3.astype(bf16),
        "w3sum": w3sum.astype(bf16),
        "actb_s": actb_s,
        "actb_t": actb_t,
        "bsp1": bsp1,
        "btv": btv,
        "csq": csq.astype(bf16),
        "yconst": np.full((1, 1), yconst, f32),
    }


def _get_nc(rows):
    key = ("nc", rows)
    if key not in _CACHE:
        _CACHE[key] = _build_nc(rows)
    return _CACHE[key]


def _run(inputs, trace=False, rows=R, ncores=NCORES):
    from concourse.bass_utils import run_bass_kernel_spmd

    full = _prep_inputs(**inputs)
    shared = {k: v for k, v in full.items() if k not in ("slabInit", "thetaT4")}
    in_maps = []
    for c in range(ncores):
        r0 = c * rows
        m = dict(shared)
        m["slabInit"] = np.ascontiguousarray(full["slabInit"][:, r0 : r0 + rows])
        m["thetaT4"] = np.ascontiguousarray(full["thetaT4"][:, r0 : r0 + rows])
        in_maps.append(m)

    nc = _get_nc(rows)
    res = run_bass_kernel_spmd(
        nc, in_maps, core_ids=list(range(ncores)), trace=trace
    )
    out = np.concatenate([res.results[c]["y"] for c in range(ncores)])
    return out, res


def kernel(**inputs):
    out, _ = _run(inputs)
    return out.astype(np.float32)
